# revision 14
# baseline (speedup 1.0000x reference)
"""Self-contained Trainium2 Bass kernel for a 3-layer MPNN (N=50000, E=800000, D=64).

Math: each layer is
    x' = relu(concat(segment_sum(x[src]@Wm+bm, dst), x) @ Wu + bu)
with self-loops added. Since the message fn is linear this folds to
    T    = x @ (Wm @ Wu[:D])                      (per-node table)
    y[v] = sum_{e: dst=v} T[src_e]                (scatter-add, no self-loop)
    x'   = relu(y + x @ (Wm@Wu[:D] + Wu[D:]) + deg*(bm@Wu[:D]) + bu)

Sharding: nodes padded to 50176 = 8*6272; core k owns nodes [6272k, 6272(k+1)),
as 98 blocks of 64 (dst side). Host sorts each core's incident edges by
(dst 64-block, src-row parity) into 128-edge chunks.

Device v2.5: T tables in bf16 packed two-rows-per-256B so the whole packed
table is int16-addressable (no half split). Gathers batched ~25 instructions
per layer on rotating SWDGE queues. Scatter-add via bf16 one-hot matmuls with
64-wide dst blocks; all one-hots are built once (layer 0) into a persistent
SBUF cache and reused by layers 1-2. One whole-table AllGather per layer
(ping-pong shared buffers). Bias and x@W2 fold into one matmul via an
augmented 66-row transposed-x ([x^T; 1; deg]) against [W2; bu; bm@Wu_top].
"""
import numpy as np
from contextlib import ExitStack

import ml_dtypes

import concourse.bass as bass
import concourse.bacc as bacc
import concourse.mybir as mybir
import concourse.tile as tile
from concourse.bass_utils import run_bass_kernel_spmd
from concourse.masks import make_identity

N = 50000
E = 800000
D = 64
NCORE = 8
P = 128
B64 = 64
PERCORE = 6272          # 98 * 64
NPAD = PERCORE * NCORE  # 50176
NB = PERCORE // B64     # 98 dst blocks of 64
F32 = mybir.dt.float32
BF16 = mybir.dt.bfloat16
I16 = mybir.dt.int16
BF = ml_dtypes.bfloat16

RPACK = (128 + NPAD) // 2   # 25152 packed (2-row) gather-table rows
GSEG = 4                    # 64-blocks per gather instruction
OHG = 8                     # one-hot chunks per DVE build


def _preprocess(edge_index):
    """Partition + sort edges by (dst-core, dst-64-block, src parity).

    Returns:
      gidx  [NCORE, P, 8*TOT]  int16 wrapped packed-table gather indices
      dstl  [NCORE, P, TOT]    bf16 dst-local-in-64blk (slot p of chunk c), -1 pad
      deg   [NPAD]             f32 in-degree + 1
      cnts2 [NB, 2]            chunks per (block, parity), shared across cores
      starts2 [NB, 2]          chunk-column starts
      TOT   total chunk columns
    """
    src = edge_index[0].astype(np.int64)
    dst = edge_index[1].astype(np.int64)
    core = dst // PERCORE
    blk = (dst % PERCORE) // B64
    loc = dst % B64
    par = src & 1
    pidx = (128 + src) >> 1          # packed row in the ext table

    deg = np.bincount(dst, minlength=NPAD).astype(np.float32) + 1.0

    order = np.lexsort((loc, par, blk, core))
    pidx_s, loc_s = pidx[order], loc[order]
    core_s, blk_s, par_s = core[order], blk[order], par[order]

    cnt = np.zeros((NCORE, NB, 2), dtype=np.int64)
    np.add.at(cnt, (core_s, blk_s, par_s), 1)
    cnts2 = (cnt + P - 1) // P
    cnts2 = cnts2.max(axis=0)  # [NB, 2]
    flat = cnts2.reshape(-1)
    starts_flat = np.concatenate([[0], np.cumsum(flat)])
    TOT = int(starts_flat[-1])
    starts2 = starts_flat[:-1].reshape(NB, 2)

    gidx = np.zeros((NCORE, P, 8 * TOT), dtype=np.int16)
    dstl = np.full((NCORE, P, TOT), -1.0, dtype=np.float32)

    run_start = np.concatenate([[0], np.cumsum(cnt.ravel())])[:-1].reshape(
        NCORE, NB, 2)
    for k in range(NCORE):
        for b in range(NB):
            for h in range(2):
                n = int(cnt[k, b, h])
                w = int(cnts2[b, h])
                if w == 0:
                    continue
                st = int(starts2[b, h])
                ridx = np.zeros((w * P,), dtype=np.int64)
                rloc = np.full((w * P,), -1.0, dtype=np.float32)
                if n:
                    s0 = int(run_start[k, b, h])
                    ridx[:n] = pidx_s[s0:s0 + n]
                    rloc[:n] = loc_s[s0:s0 + n]
                # wrapped idx: [16, w*8] -> replicate to 128 partitions
                w16 = ridx.reshape(w * 8, 16).T.astype(np.int16)
                gidx[k, :, 8 * st:8 * (st + w)] = np.tile(w16, (8, 1))
                dstl[k, :, st:st + w] = rloc.reshape(w, P).T
    return gidx, dstl.astype(BF), deg, cnts2.astype(int), starts2.astype(int), TOT


def _build(cnts2, starts2, TOT):
    nc = bacc.Bacc("TRN2", target_bir_lowering=False, debug=False,
                   num_devices=NCORE, num_swdge_queues=4)
    x_own = nc.dram_tensor("x_own", [PERCORE, D], F32, kind="ExternalInput")
    gidx_in = nc.dram_tensor("gidx", [P, 8 * TOT], I16, kind="ExternalInput")
    dst_loc = nc.dram_tensor("dst_loc", [P, TOT], BF16, kind="ExternalInput")
    deg_in = nc.dram_tensor("deg_in", [2, PERCORE], BF16, kind="ExternalInput")
    iota_in = nc.dram_tensor("iota_in", [P, OHG, B64], BF16,
                             kind="ExternalInput")
    Wm_in = nc.dram_tensor("Wm_in", [3, D, D], F32, kind="ExternalInput")
    Wu_in = nc.dram_tensor("Wu_in", [3, 2 * D, D], F32, kind="ExternalInput")
    bm_in = nc.dram_tensor("bm_in", [3, D], F32, kind="ExternalInput")
    bu_in = nc.dram_tensor("bu_in", [3, D], F32, kind="ExternalInput")
    out = nc.dram_tensor("out", [PERCORE, D], F32, kind="ExternalOutput")

    T_own = [nc.dram_tensor(f"T_own{l}", [PERCORE, D], BF16) for l in range(3)]
    # two ping-pong packed tables; layer l reads T_ext[l%2], writes (l+1)%2
    T_ext = [nc.dram_tensor(f"T_ext{i}", [RPACK, 2 * D], BF16,
                            addr_space="Shared") for i in range(2)]
    groups = [list(range(NCORE))]

    # gather segments: GSEG 64-blocks each
    gsegs = []
    for b0 in range(0, NB, GSEG):
        b1 = min(b0 + GSEG, NB)
        c0 = int(starts2[b0, 0])
        c1 = int(starts2[b1, 0]) if b1 < NB else TOT
        gsegs.append((b0, b1, c0, c1))

    def blk_chunks(b):
        """[(global chunk col, parity)] for 64-block b, in column order."""
        res = []
        for p in range(2):
            st, w = int(starts2[b, p]), int(cnts2[b, p])
            res.extend((st + j, p) for j in range(w))
        return res

    with tile.TileContext(nc) as tc, ExitStack() as ctx:
        const = ctx.enter_context(tc.tile_pool(name="const", bufs=1))
        sb = ctx.enter_context(tc.tile_pool(name="sb", bufs=4))
        gat = ctx.enter_context(tc.tile_pool(name="gat", bufs=3))
        ps_y = ctx.enter_context(tc.tile_pool(name="ps_y", bufs=3, space="PSUM"))
        ps_m = ctx.enter_context(tc.tile_pool(name="ps_m", bufs=2, space="PSUM"))
        ps_w = ctx.enter_context(tc.tile_pool(name="ps_w", bufs=1, space="PSUM"))

        ident = const.tile([P, P], F32)
        make_identity(nc, ident[:])
        identb = const.tile([B64, B64], BF16)
        make_identity(nc, identb[:])
        iota8 = const.tile([P, OHG, B64], BF16)
        nc.sync.dma_start(out=iota8[:], in_=iota_in[:])

        gidx_sb = const.tile([P, 8 * TOT], I16, tag="gidx_sb")
        nc.sync.dma_start(out=gidx_sb[:], in_=gidx_in[:])
        dst_all = const.tile([P, TOT], BF16, tag="dst_all")
        nc.sync.dma_start(out=dst_all[:], in_=dst_loc[:])

        # persistent one-hot cache: built in layer 0, reused by layers 1-2
        oh_all = const.tile([P, TOT, B64], BF16, tag="oh_all")

        # zero the 64 packed zero-rows at the head of both tables
        zrow = const.tile([64, 2 * D], BF16, tag="zrow")
        nc.vector.memset(zrow[:], 0.0)
        for i in range(2):
            nc.sync.dma_start(out=T_ext[i][0:64, :], in_=zrow[:])

        # --- per-layer weight prep: W1 = Wm@Wu_top (bf16),
        # Wb2 = [W1 + Wu_bot ; bu ; bm@Wu_top] (bf16 [66, D]) ---
        W1s, Wb2s = [], []
        for l in range(3):
            wm = const.tile([D, D], F32, tag=f"wm{l}")
            nc.sync.dma_start(out=wm[:], in_=Wm_in[l])
            wu_t = const.tile([D, D], F32, tag=f"wut{l}")
            nc.sync.dma_start(out=wu_t[:], in_=Wu_in[l, :D])
            wu_b = const.tile([D, D], F32, tag=f"wub{l}")
            nc.sync.dma_start(out=wu_b[:], in_=Wu_in[l, D:])
            wmT_ps = ps_w.tile([D, D], F32, tag="psw")
            nc.tensor.transpose(out=wmT_ps[:], in_=wm[:], identity=ident[:D, :D])
            wmT = const.tile([D, D], F32, tag=f"wmT{l}")
            nc.vector.tensor_copy(out=wmT[:], in_=wmT_ps[:])
            w1_ps = ps_w.tile([D, D], F32, tag="psw")
            nc.tensor.matmul(out=w1_ps[:], lhsT=wmT[:], rhs=wu_t[:],
                             start=True, stop=True)
            w1 = const.tile([D, D], BF16, tag=f"w1{l}")
            nc.vector.tensor_copy(out=w1[:], in_=w1_ps[:])
            wb2 = const.tile([D + 2, D], BF16, tag=f"wb2{l}")
            nc.vector.tensor_add(out=wb2[:D, :], in0=w1_ps[:], in1=wu_b[:])
            bmc = const.tile([D, 1], F32, tag=f"bmc{l}")
            nc.sync.dma_start(out=bmc[:], in_=bm_in[l][:, None])
            b1_ps = ps_w.tile([1, D], F32, tag="psw")
            nc.tensor.matmul(out=b1_ps[:], lhsT=bmc[:], rhs=wu_t[:],
                             start=True, stop=True)
            # rows 64/65 pair with the deg/ones rows of the augmented xT;
            # stage both in a partition-0-based tile (the BIR verifier
            # rejects 1-partition accesses at partition 65)
            btail = const.tile([2, D], F32, tag=f"btail{l}")
            nc.vector.tensor_copy(out=btail[:1, :], in_=b1_ps[:])
            nc.sync.dma_start(out=btail[1:2, :], in_=bu_in[l][None, :])
            nc.vector.tensor_copy(out=wb2[D:D + 2, :], in_=btail[:])
            W1s.append(w1)
            Wb2s.append(wb2)

        # persistent augmented transposed-x buffers: rows 0-63 = x^T,
        # row 64 = deg (DMA needs the aligned partition), row 65 = ones
        xT = [const.tile([D + 2, PERCORE], BF16, tag=f"xT{i}", name=f"xT{i}")
              for i in range(2)]
        for i in range(2):
            nc.sync.dma_start(out=xT[i][D:D + 2, :], in_=deg_in[:])

        # --- layer 0 table: T0 = x_own @ W1_0 (+ build xT[0]), 128-row blocks ---
        for b in range(NB // 2):
            bs, be = b * P, (b + 1) * P
            xb = sb.tile([P, D], F32, tag="xb0")
            nc.sync.dma_start(out=xb[:], in_=x_own[bs:be, :])
            xT_ps = ps_m.tile([D, P], F32, tag="psm")
            nc.tensor.transpose(out=xT_ps[:], in_=xb[:], identity=ident[:])
            nc.scalar.activation(out=xT[0][:D, bs:be], in_=xT_ps[:],
                                 func=mybir.ActivationFunctionType.Copy)
            t0_ps = ps_m.tile([P, D], F32, tag="psm")
            nc.tensor.matmul(out=t0_ps[:], lhsT=xT[0][:D, bs:be], rhs=W1s[0][:],
                             start=True, stop=True)
            t0 = sb.tile([P, D], BF16, tag="t0sb")
            nc.vector.tensor_copy(out=t0[:], in_=t0_ps[:])
            nc.sync.dma_start(out=T_own[0][bs:be, :], in_=t0[:])
        nc.gpsimd.collective_compute(
            "AllGather", mybir.AluOpType.bypass, replica_groups=groups,
            ins=[T_own[0][:]], outs=[T_ext[0][64:RPACK, :]])

        # --- 3 layers ---
        for l in range(3):
            xT_cur = xT[l % 2]
            xT_nxt = xT[(l + 1) % 2]
            tbl = T_ext[l % 2]
            for gi, (b0, b1, c0, c1) in enumerate(gsegs):
                gt = gat.tile([P, c1 - c0, 2 * D], BF16, tag="g")
                nc.gpsimd.dma_gather(
                    out_ap=gt[:],
                    in_ap=tbl[:],
                    idxs_ap=gidx_sb[:, 8 * c0:8 * c1],
                    num_idxs=(c1 - c0) * P,
                    num_idxs_reg=(c1 - c0) * P,
                    elem_size=2 * D,
                    single_packet=False,
                    queue_num=gi % 4,
                )
                if l == 0:
                    # build this segment's one-hots into the persistent cache
                    for g0 in range(c0, c1, OHG):
                        g = min(OHG, c1 - g0)
                        nc.vector.tensor_tensor(
                            out=oh_all[:, g0:g0 + g, :],
                            in0=dst_all[:, g0:g0 + g]
                                .to_broadcast([P, g, B64])[:],
                            in1=iota8[:, :g, :],
                            op=mybir.AluOpType.is_equal)
                for b in range(b0, b1):
                    bs, be = b * B64, (b + 1) * B64
                    chunks = blk_chunks(b)
                    psy = ps_y.tile([B64, D], F32, tag="psy")
                    first = True
                    for c, par in chunks:
                        nc.tensor.matmul(
                            out=psy[:], lhsT=oh_all[:, c, :],
                            rhs=gt[:, c - c0, par * D:(par + 1) * D],
                            start=first, stop=False)
                        first = False
                    nc.tensor.matmul(out=psy[:], lhsT=xT_cur[:, bs:be],
                                     rhs=Wb2s[l][:], start=first, stop=True)
                    if l == 2:
                        xout = sb.tile([B64, D], F32, tag="xout")
                        nc.scalar.activation(
                            out=xout[:], in_=psy[:],
                            func=mybir.ActivationFunctionType.Relu)
                        nc.sync.dma_start(out=out[bs:be, :], in_=xout[:])
                    else:
                        xnew = sb.tile([B64, D], BF16, tag="xnew")
                        nc.scalar.activation(
                            out=xnew[:], in_=psy[:],
                            func=mybir.ActivationFunctionType.Relu)
                        xT_ps = ps_m.tile([B64, B64], BF16, tag="psmb")
                        nc.tensor.transpose(out=xT_ps[:], in_=xnew[:],
                                            identity=identb[:])
                        nc.scalar.activation(
                            out=xT_nxt[:D, bs:be], in_=xT_ps[:],
                            func=mybir.ActivationFunctionType.Copy)
                        tn_ps = ps_m.tile([B64, D], F32, tag="psm")
                        nc.tensor.matmul(out=tn_ps[:], lhsT=xT_nxt[:D, bs:be],
                                         rhs=W1s[l + 1][:], start=True,
                                         stop=True)
                        tn = sb.tile([B64, D], BF16, tag="tnsb")
                        nc.vector.tensor_copy(out=tn[:], in_=tn_ps[:])
                        nc.sync.dma_start(out=T_own[l + 1][bs:be, :], in_=tn[:])
            if l < 2:
                nc.gpsimd.collective_compute(
                    "AllGather", mybir.AluOpType.bypass, replica_groups=groups,
                    ins=[T_own[l + 1][:]],
                    outs=[T_ext[(l + 1) % 2][64:RPACK, :]])

    nc.compile()
    return nc


def prepare(x, edge_index, Wm0, bm0, Wu0, bu0, Wm1, bm1, Wu1, bu1,
            Wm2, bm2, Wu2, bu2):
    """Returns (nc, in_maps) ready for run_bass_kernel_spmd."""
    x = np.asarray(x, dtype=np.float32)
    edge_index = np.asarray(edge_index)
    gidx, dstl, deg, cnts2, starts2, TOT = _preprocess(edge_index)

    xpad = np.zeros((NPAD, D), dtype=np.float32)
    xpad[:N] = x
    iota = np.broadcast_to(np.arange(B64, dtype=np.float32),
                           (P, OHG, B64)).astype(BF)
    Wm = np.stack([np.asarray(w, np.float32) for w in (Wm0, Wm1, Wm2)])
    Wu = np.stack([np.asarray(w, np.float32) for w in (Wu0, Wu1, Wu2)])
    bm = np.stack([np.asarray(w, np.float32) for w in (bm0, bm1, bm2)])
    bu = np.stack([np.asarray(w, np.float32) for w in (bu0, bu1, bu2)])

    in_maps = []
    for k in range(NCORE):
        deg_k = np.stack([deg[k * PERCORE:(k + 1) * PERCORE],
                          np.ones((PERCORE,), np.float32)]).astype(BF)
        in_maps.append({
            "x_own": np.ascontiguousarray(xpad[k * PERCORE:(k + 1) * PERCORE]),
            "gidx": gidx[k],
            "dst_loc": dstl[k],
            "deg_in": np.ascontiguousarray(deg_k),
            "iota_in": iota,
            "Wm_in": Wm, "Wu_in": Wu, "bm_in": bm, "bu_in": bu,
        })

    nc = _build(cnts2, starts2, TOT)
    return nc, in_maps


def kernel(**inputs):
    nc, in_maps = prepare(**inputs)
    res = run_bass_kernel_spmd(nc, in_maps, list(range(NCORE)))
    full = np.concatenate([res.results[k]["out"] for k in range(NCORE)], axis=0)
    return np.ascontiguousarray(full[:N])


# revision 15
# speedup vs baseline: 1.0422x; 1.0422x over previous
"""Self-contained Trainium2 Bass kernel for a 3-layer MPNN (N=50000, E=800000, D=64).

Math: each layer is
    x' = relu(concat(segment_sum(x[src]@Wm+bm, dst), x) @ Wu + bu)
with self-loops added. Since the message fn is linear this folds to
    T    = x @ (Wm @ Wu[:D])                      (per-node table)
    y[v] = sum_{e: dst=v} T[src_e]                (scatter-add, no self-loop)
    x'   = relu(y + x @ (Wm@Wu[:D] + Wu[D:]) + deg*(bm@Wu[:D]) + bu)

Sharding: nodes padded to 50176 = 8*6272; core k owns nodes [6272k, 6272(k+1)),
as 98 blocks of 64 (dst side). Host sorts each core's incident edges by
(dst 64-block, src-row parity) into 128-edge chunks.

Device v2.5: T tables in bf16 packed two-rows-per-256B so the whole packed
table is int16-addressable (no half split). Gathers batched ~25 instructions
per layer on rotating SWDGE queues. Scatter-add via bf16 one-hot matmuls with
64-wide dst blocks; all one-hots are built once (layer 0) into a persistent
SBUF cache and reused by layers 1-2. One whole-table AllGather per layer
(ping-pong shared buffers). Bias and x@W2 fold into one matmul via an
augmented 66-row transposed-x ([x^T; 1; deg]) against [W2; bu; bm@Wu_top].
"""
import numpy as np
from contextlib import ExitStack

import ml_dtypes

import concourse.bass as bass
import concourse.bacc as bacc
import concourse.mybir as mybir
import concourse.tile as tile
from concourse.bass_utils import run_bass_kernel_spmd
from concourse.masks import make_identity

N = 50000
E = 800000
D = 64
NCORE = 8
P = 128
B64 = 64
PERCORE = 6272          # 98 * 64
NPAD = PERCORE * NCORE  # 50176
NB = PERCORE // B64     # 98 dst blocks of 64
F32 = mybir.dt.float32
BF16 = mybir.dt.bfloat16
I16 = mybir.dt.int16
BF = ml_dtypes.bfloat16

RPACK = (128 + NPAD) // 2   # 25152 packed (2-row) gather-table rows
GSEG = 1                    # 64-blocks per gather instruction (keep the
                            # SWDGE descriptor ring under its blocking limit)
OHG = 8                     # one-hot chunks per DVE build


def _preprocess(edge_index):
    """Partition + sort edges by (dst-core, dst-64-block, src parity).

    Returns:
      gidx  [NCORE, P, 8*TOT]  int16 wrapped packed-table gather indices
      dstl  [NCORE, P, TOT]    bf16 dst-local-in-64blk (slot p of chunk c), -1 pad
      deg   [NPAD]             f32 in-degree + 1
      cnts2 [NB, 2]            chunks per (block, parity), shared across cores
      starts2 [NB, 2]          chunk-column starts
      TOT   total chunk columns
    """
    src = edge_index[0].astype(np.int64)
    dst = edge_index[1].astype(np.int64)
    core = dst // PERCORE
    blk = (dst % PERCORE) // B64
    loc = dst % B64
    par = src & 1
    pidx = (128 + src) >> 1          # packed row in the ext table

    deg = np.bincount(dst, minlength=NPAD).astype(np.float32) + 1.0

    order = np.lexsort((loc, par, blk, core))
    pidx_s, loc_s = pidx[order], loc[order]
    core_s, blk_s, par_s = core[order], blk[order], par[order]

    cnt = np.zeros((NCORE, NB, 2), dtype=np.int64)
    np.add.at(cnt, (core_s, blk_s, par_s), 1)
    cnts2 = (cnt + P - 1) // P
    cnts2 = cnts2.max(axis=0)  # [NB, 2]
    flat = cnts2.reshape(-1)
    starts_flat = np.concatenate([[0], np.cumsum(flat)])
    TOT = int(starts_flat[-1])
    starts2 = starts_flat[:-1].reshape(NB, 2)

    gidx = np.zeros((NCORE, P, 8 * TOT), dtype=np.int16)
    dstl = np.full((NCORE, P, TOT), -1.0, dtype=np.float32)

    run_start = np.concatenate([[0], np.cumsum(cnt.ravel())])[:-1].reshape(
        NCORE, NB, 2)
    for k in range(NCORE):
        for b in range(NB):
            for h in range(2):
                n = int(cnt[k, b, h])
                w = int(cnts2[b, h])
                if w == 0:
                    continue
                st = int(starts2[b, h])
                ridx = np.zeros((w * P,), dtype=np.int64)
                rloc = np.full((w * P,), -1.0, dtype=np.float32)
                if n:
                    s0 = int(run_start[k, b, h])
                    ridx[:n] = pidx_s[s0:s0 + n]
                    rloc[:n] = loc_s[s0:s0 + n]
                # wrapped idx: [16, w*8] -> replicate to 128 partitions
                w16 = ridx.reshape(w * 8, 16).T.astype(np.int16)
                gidx[k, :, 8 * st:8 * (st + w)] = np.tile(w16, (8, 1))
                dstl[k, :, st:st + w] = rloc.reshape(w, P).T
    return gidx, dstl.astype(BF), deg, cnts2.astype(int), starts2.astype(int), TOT


def _build(cnts2, starts2, TOT):
    nc = bacc.Bacc("TRN2", target_bir_lowering=False, debug=False,
                   num_devices=NCORE, num_swdge_queues=4)
    x_own = nc.dram_tensor("x_own", [PERCORE, D], F32, kind="ExternalInput")
    gidx_in = nc.dram_tensor("gidx", [P, 8 * TOT], I16, kind="ExternalInput")
    dst_loc = nc.dram_tensor("dst_loc", [P, TOT], BF16, kind="ExternalInput")
    deg_in = nc.dram_tensor("deg_in", [2, PERCORE], BF16, kind="ExternalInput")
    iota_in = nc.dram_tensor("iota_in", [P, OHG, B64], BF16,
                             kind="ExternalInput")
    Wm_in = nc.dram_tensor("Wm_in", [3, D, D], F32, kind="ExternalInput")
    Wu_in = nc.dram_tensor("Wu_in", [3, 2 * D, D], F32, kind="ExternalInput")
    bm_in = nc.dram_tensor("bm_in", [3, D], F32, kind="ExternalInput")
    bu_in = nc.dram_tensor("bu_in", [3, D], F32, kind="ExternalInput")
    out = nc.dram_tensor("out", [PERCORE, D], F32, kind="ExternalOutput")

    T_own = [nc.dram_tensor(f"T_own{l}", [PERCORE, D], BF16) for l in range(3)]
    # two ping-pong packed tables; layer l reads T_ext[l%2], writes (l+1)%2
    T_ext = [nc.dram_tensor(f"T_ext{i}", [RPACK, 2 * D], BF16,
                            addr_space="Shared") for i in range(2)]
    groups = [list(range(NCORE))]

    # gather segments: GSEG 64-blocks each
    gsegs = []
    for b0 in range(0, NB, GSEG):
        b1 = min(b0 + GSEG, NB)
        c0 = int(starts2[b0, 0])
        c1 = int(starts2[b1, 0]) if b1 < NB else TOT
        gsegs.append((b0, b1, c0, c1))

    def blk_chunks(b):
        """[(global chunk col, parity)] for 64-block b, in column order."""
        res = []
        for p in range(2):
            st, w = int(starts2[b, p]), int(cnts2[b, p])
            res.extend((st + j, p) for j in range(w))
        return res

    with tile.TileContext(nc) as tc, ExitStack() as ctx:
        const = ctx.enter_context(tc.tile_pool(name="const", bufs=1))
        sb = ctx.enter_context(tc.tile_pool(name="sb", bufs=4))
        gat = ctx.enter_context(tc.tile_pool(name="gat", bufs=3))
        ps_y = ctx.enter_context(tc.tile_pool(name="ps_y", bufs=3, space="PSUM"))
        ps_m = ctx.enter_context(tc.tile_pool(name="ps_m", bufs=2, space="PSUM"))
        ps_w = ctx.enter_context(tc.tile_pool(name="ps_w", bufs=1, space="PSUM"))

        ident = const.tile([P, P], F32)
        make_identity(nc, ident[:])
        identb = const.tile([B64, B64], BF16)
        make_identity(nc, identb[:])
        iota8 = const.tile([P, OHG, B64], BF16)
        nc.sync.dma_start(out=iota8[:], in_=iota_in[:])

        gidx_sb = const.tile([P, 8 * TOT], I16, tag="gidx_sb")
        nc.sync.dma_start(out=gidx_sb[:], in_=gidx_in[:])
        dst_all = const.tile([P, TOT], BF16, tag="dst_all")
        nc.sync.dma_start(out=dst_all[:], in_=dst_loc[:])

        # persistent one-hot cache: built in layer 0, reused by layers 1-2
        oh_all = const.tile([P, TOT, B64], BF16, tag="oh_all")

        # zero the 64 packed zero-rows at the head of both tables
        zrow = const.tile([64, 2 * D], BF16, tag="zrow")
        nc.vector.memset(zrow[:], 0.0)
        for i in range(2):
            nc.sync.dma_start(out=T_ext[i][0:64, :], in_=zrow[:])

        # --- per-layer weight prep: W1 = Wm@Wu_top (bf16),
        # Wb2 = [W1 + Wu_bot ; bu ; bm@Wu_top] (bf16 [66, D]) ---
        W1s, Wb2s = [], []
        for l in range(3):
            wm = const.tile([D, D], F32, tag=f"wm{l}")
            nc.sync.dma_start(out=wm[:], in_=Wm_in[l])
            wu_t = const.tile([D, D], F32, tag=f"wut{l}")
            nc.sync.dma_start(out=wu_t[:], in_=Wu_in[l, :D])
            wu_b = const.tile([D, D], F32, tag=f"wub{l}")
            nc.sync.dma_start(out=wu_b[:], in_=Wu_in[l, D:])
            wmT_ps = ps_w.tile([D, D], F32, tag="psw")
            nc.tensor.transpose(out=wmT_ps[:], in_=wm[:], identity=ident[:D, :D])
            wmT = const.tile([D, D], F32, tag=f"wmT{l}")
            nc.vector.tensor_copy(out=wmT[:], in_=wmT_ps[:])
            w1_ps = ps_w.tile([D, D], F32, tag="psw")
            nc.tensor.matmul(out=w1_ps[:], lhsT=wmT[:], rhs=wu_t[:],
                             start=True, stop=True)
            w1 = const.tile([D, D], BF16, tag=f"w1{l}")
            nc.vector.tensor_copy(out=w1[:], in_=w1_ps[:])
            wb2 = const.tile([D + 2, D], BF16, tag=f"wb2{l}")
            nc.vector.tensor_add(out=wb2[:D, :], in0=w1_ps[:], in1=wu_b[:])
            bmc = const.tile([D, 1], F32, tag=f"bmc{l}")
            nc.sync.dma_start(out=bmc[:], in_=bm_in[l][:, None])
            b1_ps = ps_w.tile([1, D], F32, tag="psw")
            nc.tensor.matmul(out=b1_ps[:], lhsT=bmc[:], rhs=wu_t[:],
                             start=True, stop=True)
            # rows 64/65 pair with the deg/ones rows of the augmented xT;
            # stage both in a partition-0-based tile (the BIR verifier
            # rejects 1-partition accesses at partition 65)
            btail = const.tile([2, D], F32, tag=f"btail{l}")
            nc.vector.tensor_copy(out=btail[:1, :], in_=b1_ps[:])
            nc.sync.dma_start(out=btail[1:2, :], in_=bu_in[l][None, :])
            nc.vector.tensor_copy(out=wb2[D:D + 2, :], in_=btail[:])
            W1s.append(w1)
            Wb2s.append(wb2)

        # persistent augmented transposed-x buffers: rows 0-63 = x^T,
        # row 64 = deg (DMA needs the aligned partition), row 65 = ones
        xT = [const.tile([D + 2, PERCORE], BF16, tag=f"xT{i}", name=f"xT{i}")
              for i in range(2)]
        for i in range(2):
            nc.sync.dma_start(out=xT[i][D:D + 2, :], in_=deg_in[:])

        # --- layer 0 table: T0 = x_own @ W1_0 (+ build xT[0]), 128-row blocks ---
        for b in range(NB // 2):
            bs, be = b * P, (b + 1) * P
            xb = sb.tile([P, D], F32, tag="xb0")
            nc.sync.dma_start(out=xb[:], in_=x_own[bs:be, :])
            xT_ps = ps_m.tile([D, P], F32, tag="psm")
            nc.tensor.transpose(out=xT_ps[:], in_=xb[:], identity=ident[:])
            nc.scalar.activation(out=xT[0][:D, bs:be], in_=xT_ps[:],
                                 func=mybir.ActivationFunctionType.Copy)
            t0_ps = ps_m.tile([P, D], F32, tag="psm")
            nc.tensor.matmul(out=t0_ps[:], lhsT=xT[0][:D, bs:be], rhs=W1s[0][:],
                             start=True, stop=True)
            t0 = sb.tile([P, D], BF16, tag="t0sb")
            nc.vector.tensor_copy(out=t0[:], in_=t0_ps[:])
            nc.sync.dma_start(out=T_own[0][bs:be, :], in_=t0[:])
        nc.gpsimd.collective_compute(
            "AllGather", mybir.AluOpType.bypass, replica_groups=groups,
            ins=[T_own[0][:]], outs=[T_ext[0][64:RPACK, :]])

        # --- 3 layers ---
        for l in range(3):
            xT_cur = xT[l % 2]
            xT_nxt = xT[(l + 1) % 2]
            tbl = T_ext[l % 2]
            for gi, (b0, b1, c0, c1) in enumerate(gsegs):
                gt = gat.tile([P, c1 - c0, 2 * D], BF16, tag="g")
                nc.gpsimd.dma_gather(
                    out_ap=gt[:],
                    in_ap=tbl[:],
                    idxs_ap=gidx_sb[:, 8 * c0:8 * c1],
                    num_idxs=(c1 - c0) * P,
                    num_idxs_reg=(c1 - c0) * P,
                    elem_size=2 * D,
                    single_packet=False,
                    queue_num=gi % 4,
                )
                if l == 0:
                    # build this segment's one-hots into the persistent cache
                    for g0 in range(c0, c1, OHG):
                        g = min(OHG, c1 - g0)
                        nc.vector.tensor_tensor(
                            out=oh_all[:, g0:g0 + g, :],
                            in0=dst_all[:, g0:g0 + g]
                                .to_broadcast([P, g, B64])[:],
                            in1=iota8[:, :g, :],
                            op=mybir.AluOpType.is_equal)
                for b in range(b0, b1):
                    bs, be = b * B64, (b + 1) * B64
                    chunks = blk_chunks(b)
                    psy = ps_y.tile([B64, D], F32, tag="psy")
                    first = True
                    for c, par in chunks:
                        nc.tensor.matmul(
                            out=psy[:], lhsT=oh_all[:, c, :],
                            rhs=gt[:, c - c0, par * D:(par + 1) * D],
                            start=first, stop=False)
                        first = False
                    nc.tensor.matmul(out=psy[:], lhsT=xT_cur[:, bs:be],
                                     rhs=Wb2s[l][:], start=first, stop=True)
                    if l == 2:
                        xout = sb.tile([B64, D], F32, tag="xout")
                        nc.scalar.activation(
                            out=xout[:], in_=psy[:],
                            func=mybir.ActivationFunctionType.Relu)
                        nc.sync.dma_start(out=out[bs:be, :], in_=xout[:])
                    else:
                        xnew = sb.tile([B64, D], BF16, tag="xnew")
                        nc.scalar.activation(
                            out=xnew[:], in_=psy[:],
                            func=mybir.ActivationFunctionType.Relu)
                        xT_ps = ps_m.tile([B64, B64], BF16, tag="psmb")
                        nc.tensor.transpose(out=xT_ps[:], in_=xnew[:],
                                            identity=identb[:])
                        nc.scalar.activation(
                            out=xT_nxt[:D, bs:be], in_=xT_ps[:],
                            func=mybir.ActivationFunctionType.Copy)
                        tn_ps = ps_m.tile([B64, D], F32, tag="psm")
                        nc.tensor.matmul(out=tn_ps[:], lhsT=xT_nxt[:D, bs:be],
                                         rhs=W1s[l + 1][:], start=True,
                                         stop=True)
                        tn = sb.tile([B64, D], BF16, tag="tnsb")
                        nc.vector.tensor_copy(out=tn[:], in_=tn_ps[:])
                        nc.sync.dma_start(out=T_own[l + 1][bs:be, :], in_=tn[:])
            if l < 2:
                nc.gpsimd.collective_compute(
                    "AllGather", mybir.AluOpType.bypass, replica_groups=groups,
                    ins=[T_own[l + 1][:]],
                    outs=[T_ext[(l + 1) % 2][64:RPACK, :]])

    nc.compile()
    return nc


def prepare(x, edge_index, Wm0, bm0, Wu0, bu0, Wm1, bm1, Wu1, bu1,
            Wm2, bm2, Wu2, bu2):
    """Returns (nc, in_maps) ready for run_bass_kernel_spmd."""
    x = np.asarray(x, dtype=np.float32)
    edge_index = np.asarray(edge_index)
    gidx, dstl, deg, cnts2, starts2, TOT = _preprocess(edge_index)

    xpad = np.zeros((NPAD, D), dtype=np.float32)
    xpad[:N] = x
    iota = np.broadcast_to(np.arange(B64, dtype=np.float32),
                           (P, OHG, B64)).astype(BF)
    Wm = np.stack([np.asarray(w, np.float32) for w in (Wm0, Wm1, Wm2)])
    Wu = np.stack([np.asarray(w, np.float32) for w in (Wu0, Wu1, Wu2)])
    bm = np.stack([np.asarray(w, np.float32) for w in (bm0, bm1, bm2)])
    bu = np.stack([np.asarray(w, np.float32) for w in (bu0, bu1, bu2)])

    in_maps = []
    for k in range(NCORE):
        deg_k = np.stack([deg[k * PERCORE:(k + 1) * PERCORE],
                          np.ones((PERCORE,), np.float32)]).astype(BF)
        in_maps.append({
            "x_own": np.ascontiguousarray(xpad[k * PERCORE:(k + 1) * PERCORE]),
            "gidx": gidx[k],
            "dst_loc": dstl[k],
            "deg_in": np.ascontiguousarray(deg_k),
            "iota_in": iota,
            "Wm_in": Wm, "Wu_in": Wu, "bm_in": bm, "bu_in": bu,
        })

    nc = _build(cnts2, starts2, TOT)
    return nc, in_maps


def kernel(**inputs):
    nc, in_maps = prepare(**inputs)
    res = run_bass_kernel_spmd(nc, in_maps, list(range(NCORE)))
    full = np.concatenate([res.results[k]["out"] for k in range(NCORE)], axis=0)
    return np.ascontiguousarray(full[:N])


# revision 24
# speedup vs baseline: 1.6147x; 1.5494x over previous
"""Self-contained Trainium2 Bass kernel for a 3-layer MPNN (N=50000, E=800000, D=64).

Math: each layer is
    x' = relu(concat(segment_sum(x[src]@Wm+bm, dst), x) @ Wu + bu)
with self-loops added. Since the message fn is linear this folds to
    T    = x @ (Wm @ Wu[:D])                      (per-node table)
    y[v] = sum_{e: dst=v} T[src_e]                (scatter-add, no self-loop)
    x'   = relu(y + x @ (Wm@Wu[:D] + Wu[D:]) + deg*(bm@Wu[:D]) + bu)

Sharding: nodes padded to 50176 = 8*6272; core k owns nodes [6272k, 6272(k+1)),
as 49 blocks of 128. Host sorts each core's incident edges by (dst block,
src-row parity) into 128-edge chunks.

Device v2 (vs baseline): T tables in bf16 packed two-rows-per-256B so the
whole 25152-row packed table is addressable by int16 gather indices (no
half split). Gathers are batched ~10 instructions per layer (5-block
segments) to amortize SWDGE descriptor-gen on GpSimd. Scatter-add uses
bf16 one-hot matmuls (4x PE throughput), with one-hots built 8 chunks per
DVE instruction. The next layer's T table AllGather is split into 4
segments fired as soon as their blocks complete, overlapping the
collective with compute; T tables ping-pong between two shared buffers.
"""
import numpy as np
from contextlib import ExitStack

import ml_dtypes

import concourse.bass as bass
import concourse.bacc as bacc
import concourse.mybir as mybir
import concourse.tile as tile
from concourse.bass_utils import run_bass_kernel_spmd
from concourse.masks import make_identity

N = 50000
E = 800000
D = 64
NCORE = 8
P = 128
PERCORE = 6272          # 49 * 128
NPAD = PERCORE * NCORE  # 50176
NBLK = PERCORE // P     # 49
F32 = mybir.dt.float32
BF16 = mybir.dt.bfloat16
I16 = mybir.dt.int16
BF = ml_dtypes.bfloat16

# AllGather segments: blocks per segment and derived row offsets
SEG_BLKS = [13, 12, 12, 12]
SEG_BSTART = [0, 13, 25, 37]               # first block of each segment
SEG_ROWS = [b * P for b in SEG_BLKS]       # per-core rows per segment
# ext row-space: 128 zero rows, then seg s holds 8*SEG_ROWS[s] rows
SEG_EXT_START = [128, 13440, 25728, 38016]
RPACK = (128 + NPAD) // 2                  # 25152 packed (2-row) table rows
GSEG = 2                                   # blocks per gather instruction
OHG = 8                                    # one-hot chunks per DVE build


def _ext_row(src):
    """Map global node id -> row in the zero-headed table AllGather builds."""
    return 128 + src


def _preprocess(edge_index):
    """Partition + sort edges by (dst-core, dst-block, src-row parity).

    Returns:
      gidx  [NCORE, P, 8*TOT]  int16 wrapped packed-table gather indices
      dstl  [NCORE, P, TOT]    bf16 dst-local (edge slot p of chunk col c), -1 pad
      deg   [NPAD]             f32 in-degree + 1
      cnts2 [NBLK, 2]          chunks per (block, parity), shared across cores
      starts2 [NBLK, 2]        chunk-column starts
      TOT   total chunk columns
    """
    src = edge_index[0].astype(np.int64)
    dst = edge_index[1].astype(np.int64)
    core = dst // PERCORE
    blk = (dst % PERCORE) // P
    loc = dst % P

    ext = _ext_row(src)
    pidx = ext >> 1
    par = (ext & 1).astype(np.int64)

    deg = np.bincount(dst, minlength=NPAD).astype(np.float32) + 1.0

    order = np.lexsort((loc, par, blk, core))
    pidx_s, loc_s = pidx[order], loc[order]
    core_s, blk_s, par_s = core[order], blk[order], par[order]

    cnt = np.zeros((NCORE, NBLK, 2), dtype=np.int64)
    np.add.at(cnt, (core_s, blk_s, par_s), 1)
    cnts2 = (cnt + P - 1) // P
    cnts2 = cnts2.max(axis=0)  # [NBLK, 2]
    flat = cnts2.reshape(-1)
    starts_flat = np.concatenate([[0], np.cumsum(flat)])
    TOT = int(starts_flat[-1])
    starts2 = starts_flat[:-1].reshape(NBLK, 2)

    gidx = np.zeros((NCORE, P, 8 * TOT), dtype=np.int16)
    dstl = np.full((NCORE, P, TOT), -1.0, dtype=np.float32)

    run_start = np.concatenate([[0], np.cumsum(cnt.ravel())])[:-1].reshape(
        NCORE, NBLK, 2)
    for k in range(NCORE):
        for b in range(NBLK):
            for h in range(2):
                n = int(cnt[k, b, h])
                w = int(cnts2[b, h])
                if w == 0:
                    continue
                st = int(starts2[b, h])
                ridx = np.zeros((w * P,), dtype=np.int64)
                rloc = np.full((w * P,), -1.0, dtype=np.float32)
                if n:
                    s0 = int(run_start[k, b, h])
                    ridx[:n] = pidx_s[s0:s0 + n]
                    rloc[:n] = loc_s[s0:s0 + n]
                # wrapped idx: [16, w*8] -> replicate to 128 partitions
                w16 = ridx.reshape(w * 8, 16).T.astype(np.int16)
                gidx[k, :, 8 * st:8 * (st + w)] = np.tile(w16, (8, 1))
                dstl[k, :, st:st + w] = rloc.reshape(w, P).T
    return gidx, dstl.astype(BF), deg, cnts2.astype(int), starts2.astype(int), TOT


def _build(cnts2, starts2, TOT):
    nc = bacc.Bacc("TRN2", target_bir_lowering=False, debug=False,
                   num_devices=NCORE, num_swdge_queues=4)
    x_own = nc.dram_tensor("x_own", [PERCORE, D], F32, kind="ExternalInput")
    gidx_in = nc.dram_tensor("gidx", [P, 8 * TOT], I16, kind="ExternalInput")
    dst_loc = nc.dram_tensor("dst_loc", [P, TOT], BF16, kind="ExternalInput")
    deg2 = nc.dram_tensor("deg2", [2, PERCORE], BF16, kind="ExternalInput")
    iota_in = nc.dram_tensor("iota_in", [P, OHG, P], BF16, kind="ExternalInput")
    Wm_in = nc.dram_tensor("Wm_in", [3, D, D], F32, kind="ExternalInput")
    Wu_in = nc.dram_tensor("Wu_in", [3, 2 * D, D], F32, kind="ExternalInput")
    bm_in = nc.dram_tensor("bm_in", [3, D], F32, kind="ExternalInput")
    bu_in = nc.dram_tensor("bu_in", [3, D], F32, kind="ExternalInput")
    out = nc.dram_tensor("out", [PERCORE, D], F32, kind="ExternalOutput")

    T_own = [nc.dram_tensor(f"T_own{l}", [PERCORE, D], BF16) for l in range(3)]
    # two ping-pong packed tables; layer l reads T_ext[l%2], writes (l+1)%2
    T_ext = [nc.dram_tensor(f"T_ext{i}", [RPACK, 2 * D], BF16,
                            addr_space="Shared") for i in range(2)]
    groups = [list(range(NCORE))]

    def blk_chunks(b):
        """[(global chunk col, parity)] for block b, in column order."""
        out = []
        for p in range(2):
            st, w = int(starts2[b, p]), int(cnts2[b, p])
            out.extend((st + j, p) for j in range(w))
        return out

    with tile.TileContext(nc) as tc, ExitStack() as ctx:
        const = ctx.enter_context(tc.tile_pool(name="const", bufs=1))
        sb = ctx.enter_context(tc.tile_pool(name="sb", bufs=4))
        gat = ctx.enter_context(tc.tile_pool(name="gat", bufs=8))
        oneh = ctx.enter_context(tc.tile_pool(name="oneh", bufs=6))
        ps_y = ctx.enter_context(tc.tile_pool(name="ps_y", bufs=3, space="PSUM"))
        ps_m = ctx.enter_context(tc.tile_pool(name="ps_m", bufs=2, space="PSUM"))
        ps_w = ctx.enter_context(tc.tile_pool(name="ps_w", bufs=1, space="PSUM"))

        ident = const.tile([P, P], F32)
        make_identity(nc, ident[:])
        identb = const.tile([P, P], BF16)
        make_identity(nc, identb[:])
        iota8 = const.tile([P, OHG, P], BF16)
        nc.sync.dma_start(out=iota8[:], in_=iota_in[:])

        gidx_sb = const.tile([P, 8 * TOT], I16, tag="gidx_sb")
        nc.sync.dma_start(out=gidx_sb[:], in_=gidx_in[:])
        dst_all = const.tile([P, TOT], BF16, tag="dst_all")
        nc.sync.dma_start(out=dst_all[:], in_=dst_loc[:])
        degt = const.tile([2, PERCORE], BF16, tag="degt")
        nc.sync.dma_start(out=degt[:], in_=deg2[:])

        # zero the 64 packed zero-rows at the head of both tables
        zrow = const.tile([64, 2 * D], BF16, tag="zrow")
        nc.vector.memset(zrow[:], 0.0)
        for i in range(2):
            nc.sync.dma_start(out=T_ext[i][0:64, :], in_=zrow[:])

        # --- per-layer weight prep: W1 = Wm@Wu_top, W2 = W1 + Wu_bot,
        # bias_rhs = [bm@Wu_top ; bu]; all cast to bf16 ---
        W1s, W2s, biasr = [], [], []
        for l in range(3):
            wm = const.tile([D, D], F32, tag=f"wm{l}")
            nc.sync.dma_start(out=wm[:], in_=Wm_in[l])
            wu_t = const.tile([D, D], F32, tag=f"wut{l}")
            nc.sync.dma_start(out=wu_t[:], in_=Wu_in[l, :D])
            wu_b = const.tile([D, D], F32, tag=f"wub{l}")
            nc.sync.dma_start(out=wu_b[:], in_=Wu_in[l, D:])
            wmT_ps = ps_w.tile([D, D], F32, tag="psw")
            nc.tensor.transpose(out=wmT_ps[:], in_=wm[:], identity=ident[:D, :D])
            wmT = const.tile([D, D], F32, tag=f"wmT{l}")
            nc.vector.tensor_copy(out=wmT[:], in_=wmT_ps[:])
            w1_ps = ps_w.tile([D, D], F32, tag="psw")
            nc.tensor.matmul(out=w1_ps[:], lhsT=wmT[:], rhs=wu_t[:],
                             start=True, stop=True)
            w1 = const.tile([D, D], BF16, tag=f"w1{l}")
            nc.vector.tensor_copy(out=w1[:], in_=w1_ps[:])
            w2 = const.tile([D, D], BF16, tag=f"w2{l}")
            nc.vector.tensor_add(out=w2[:], in0=w1_ps[:], in1=wu_b[:])
            bmc = const.tile([D, 1], F32, tag=f"bmc{l}")
            nc.sync.dma_start(out=bmc[:], in_=bm_in[l][:, None])
            b1_ps = ps_w.tile([1, D], F32, tag="psw")
            nc.tensor.matmul(out=b1_ps[:], lhsT=bmc[:], rhs=wu_t[:],
                             start=True, stop=True)
            buf32 = const.tile([2, D], F32, tag=f"brf{l}")
            nc.vector.tensor_copy(out=buf32[:1, :], in_=b1_ps[:])
            nc.sync.dma_start(out=buf32[1:2, :], in_=bu_in[l][None, :])
            br = const.tile([2, D], BF16, tag=f"br{l}")
            nc.vector.tensor_copy(out=br[:], in_=buf32[:])
            W1s.append(w1)
            W2s.append(w2)
            biasr.append(br)

        # persistent transposed-x buffers (bf16, ping-pong across layers)
        xT = [const.tile([D, PERCORE], BF16, tag=f"xT{i}", name=f"xT{i}")
              for i in range(2)]

        def fire_collective(l_next):
            nc.gpsimd.collective_compute(
                "AllGather", mybir.AluOpType.bypass, replica_groups=groups,
                ins=[T_own[l_next][:]],
                outs=[T_ext[l_next % 2][64:RPACK, :]])

        # --- layer 0 table: T0 = x_own @ W1_0 (+ build xT[0]) ---
        for b in range(NBLK):
            bs, be = b * P, (b + 1) * P
            xb = sb.tile([P, D], F32, tag="xb0")
            nc.sync.dma_start(out=xb[:], in_=x_own[bs:be, :])
            xT_ps = ps_m.tile([D, P], F32, tag="psm")
            nc.tensor.transpose(out=xT_ps[:], in_=xb[:], identity=ident[:])
            nc.scalar.activation(out=xT[0][:, bs:be], in_=xT_ps[:],
                                 func=mybir.ActivationFunctionType.Copy)
            t0_ps = ps_m.tile([P, D], F32, tag="psm")
            nc.tensor.matmul(out=t0_ps[:], lhsT=xT[0][:, bs:be], rhs=W1s[0][:],
                             start=True, stop=True)
            t0 = sb.tile([P, D], BF16, tag="t0sb")
            nc.vector.tensor_copy(out=t0[:], in_=t0_ps[:])
            nc.sync.dma_start(out=T_own[0][bs:be, :], in_=t0[:])
        fire_collective(0)

        # --- 3 layers ---
        for l in range(3):
            xT_cur = xT[l % 2]
            xT_nxt = xT[(l + 1) % 2]
            tbl = T_ext[l % 2]
            for b in range(NBLK):
                bs, be = b * P, (b + 1) * P
                # one gather per (block, parity) run: ~1150 descriptors,
                # small enough that SWDGE descgen never blocks on ring space
                gts = {}
                for par in range(2):
                    st, w = int(starts2[b, par]), int(cnts2[b, par])
                    if w == 0:
                        continue
                    gt = gat.tile([P, w, 2 * D], BF16, tag="g")
                    nc.gpsimd.dma_gather(
                        out_ap=gt[:],
                        in_ap=tbl[:],
                        idxs_ap=gidx_sb[:, 8 * st:8 * (st + w)],
                        num_idxs=w * P,
                        num_idxs_reg=w * P,
                        elem_size=2 * D,
                        single_packet=False,
                        queue_num=(b * 2 + par) % 4,
                    )
                    gts[par] = (gt, st)
                chunks = blk_chunks(b)
                psy = ps_y.tile([P, D], F32, tag="psy")
                first = True
                oh = None
                for j, (c, par) in enumerate(chunks):
                    if j % OHG == 0:
                        g = min(OHG, len(chunks) - j)
                        st = chunks[j][0]
                        oh = oneh.tile([P, g, P], BF16, tag="oh")
                        nc.vector.tensor_tensor(
                            out=oh[:],
                            in0=dst_all[:, st:st + g]
                                .to_broadcast([P, g, P])[:],
                            in1=iota8[:, :g, :],
                            op=mybir.AluOpType.is_equal)
                    gt, gst = gts[par]
                    nc.tensor.matmul(
                        out=psy[:], lhsT=oh[:, j % OHG, :],
                        rhs=gt[:, c - gst, par * D:(par + 1) * D],
                        start=first, stop=False)
                    first = False
                nc.tensor.matmul(out=psy[:], lhsT=xT_cur[:, bs:be],
                                 rhs=W2s[l][:], start=first, stop=False)
                nc.tensor.matmul(out=psy[:], lhsT=degt[:, bs:be],
                                 rhs=biasr[l][:], start=False, stop=True)
                if l == 2:
                    xout = sb.tile([P, D], F32, tag="xout")
                    nc.scalar.activation(
                        out=xout[:], in_=psy[:],
                        func=mybir.ActivationFunctionType.Relu)
                    nc.sync.dma_start(out=out[bs:be, :], in_=xout[:])
                else:
                    xnew = sb.tile([P, D], BF16, tag="xnew")
                    nc.scalar.activation(
                        out=xnew[:], in_=psy[:],
                        func=mybir.ActivationFunctionType.Relu)
                    xT_ps = ps_m.tile([D, P], BF16, tag="psmb")
                    nc.tensor.transpose(out=xT_ps[:], in_=xnew[:],
                                        identity=identb[:])
                    nc.scalar.activation(
                        out=xT_nxt[:, bs:be], in_=xT_ps[:],
                        func=mybir.ActivationFunctionType.Copy)
                    tn_ps = ps_m.tile([P, D], F32, tag="psm")
                    nc.tensor.matmul(out=tn_ps[:], lhsT=xT_nxt[:, bs:be],
                                     rhs=W1s[l + 1][:], start=True,
                                     stop=True)
                    tn = sb.tile([P, D], BF16, tag="tnsb")
                    nc.vector.tensor_copy(out=tn[:], in_=tn_ps[:])
                    nc.sync.dma_start(out=T_own[l + 1][bs:be, :], in_=tn[:])
            if l < 2:
                fire_collective(l + 1)

    nc.compile()
    return nc


def prepare(x, edge_index, Wm0, bm0, Wu0, bu0, Wm1, bm1, Wu1, bu1,
            Wm2, bm2, Wu2, bu2):
    """Returns (nc, in_maps) ready for run_bass_kernel_spmd."""
    x = np.asarray(x, dtype=np.float32)
    edge_index = np.asarray(edge_index)
    gidx, dstl, deg, cnts2, starts2, TOT = _preprocess(edge_index)

    xpad = np.zeros((NPAD, D), dtype=np.float32)
    xpad[:N] = x
    iota = np.broadcast_to(np.arange(P, dtype=np.float32),
                           (P, OHG, P)).astype(BF)
    Wm = np.stack([np.asarray(w, np.float32) for w in (Wm0, Wm1, Wm2)])
    Wu = np.stack([np.asarray(w, np.float32) for w in (Wu0, Wu1, Wu2)])
    bm = np.stack([np.asarray(w, np.float32) for w in (bm0, bm1, bm2)])
    bu = np.stack([np.asarray(w, np.float32) for w in (bu0, bu1, bu2)])

    in_maps = []
    for k in range(NCORE):
        deg_k = deg[k * PERCORE:(k + 1) * PERCORE]
        deg2v = np.stack([deg_k, np.ones_like(deg_k)], axis=0).astype(BF)
        in_maps.append({
            "x_own": np.ascontiguousarray(xpad[k * PERCORE:(k + 1) * PERCORE]),
            "gidx": gidx[k],
            "dst_loc": dstl[k],
            "deg2": np.ascontiguousarray(deg2v),
            "iota_in": iota,
            "Wm_in": Wm, "Wu_in": Wu, "bm_in": bm, "bu_in": bu,
        })

    nc = _build(cnts2, starts2, TOT)
    return nc, in_maps


def kernel(**inputs):
    nc, in_maps = prepare(**inputs)
    res = run_bass_kernel_spmd(nc, in_maps, list(range(NCORE)))
    full = np.concatenate([res.results[k]["out"] for k in range(NCORE)], axis=0)
    return np.ascontiguousarray(full[:N])
